# revision 7
# baseline (speedup 1.0000x reference)
"""Trainium2 Bass kernel for nn_AttentionBlock (GroupNorm + MHA + proj + residual).

Problem: x[8, 512, 32, 32] fp32; GroupNorm(32 groups) -> qkv (1x1 conv) ->
8-head attention over 1024 spatial positions -> proj -> residual.

Sharding: data-parallel over batch. 8 batch elements -> 8 NeuronCores,
one SPMD NEFF. No collectives.

Per-core design (x as [c=512, n=1024]):
  - GroupNorm: per-channel sum/sumsq (DVE reduce + ACT square-accum), group
    reduce via a tiny matmul with a host-built selection matrix (1/16384
    entries), rsqrt via exp(-0.5*ln(var+eps)), per-channel affine via
    tensor_scalar. gamma/beta are folded into qkv weights/biases on host.
  - qkv: q,k as [512, 1024] (heads on partitions); v computed TRANSPOSED
    (v^T [1024, 512]) directly by swapping matmul operands, augmented with a
    ones column per head for the softmax denominator.
  - attention per head: scores^T = k_h^T q_h (K=64, head pairs at partition
    bases 0/64 row-pack on the PE); probs = exp(scores^T) via ACT (the 1/8
    scale is folded into Wq on host; no max-subtraction needed since
    |scores| < ~10); av+denominator in one matmul with v^T|ones as the
    stationary operand; normalization by broadcast reciprocal via small
    DRAM round-trips (DMA engines) and one tensor_tensor multiply.
  - proj: K=128 matmuls on the assembled, normalized A [512, 1024];
    v-bias and proj-bias folded into one per-channel bias on host;
    residual added during eviction.
  - Matmuls run in float32r (full-rate fp32 on the PE); operands are
    "rounded" either by declaring DRAM inputs float32r or by DVE/ACT
    eviction into float32r tiles.
"""
import sys

sys.path.insert(0, "/opt/trn_rl_repo")

import numpy as np

import concourse.bass as bass
import concourse.bacc as bacc
import concourse.tile as tile
from concourse import mybir
from concourse.bass_utils import run_bass_kernel_spmd

F32 = mybir.dt.float32
F32R = mybir.dt.float32r
AX = mybir.AxisListType
OP = mybir.AluOpType
AF = mybir.ActivationFunctionType

C = 512          # channels
N = 1024         # spatial positions (32*32)
HEADS = 8
HD = 64          # head dim
G = 32           # groups
GSZ = 16         # channels per group
EPS = 1e-6
NC4 = 4          # channel chunks of 128
NM8 = 8          # spatial chunks of 128
WQK = 1024       # wpack col offsets
WV0 = 1024
WP0 = 1536
GS0 = 2048
WPACK_COLS = 2080


def build_nc():
    nc = bacc.Bacc(None)
    x = nc.declare_dram_parameter("x", [C, N], F32, isOutput=False)
    wpack = nc.declare_dram_parameter("wpack", [C, WPACK_COLS], F32R, isOutput=False)
    gselT = nc.declare_dram_parameter("gselT", [G, C], F32, isOutput=False)
    bpack = nc.declare_dram_parameter("bpack", [128, 12], F32, isOutput=False)
    y = nc.declare_dram_parameter("y", [C, N], F32, isOutput=True)

    dall = nc.dram_tensor("dall", [HEADS, N], F32)
    rdram = nc.dram_tensor("rdram", [HEADS, N], F32)

    with tile.TileContext(nc) as tc:
        with (
            tc.tile_pool(name="const", bufs=1) as const,
            tc.tile_pool(name="main", bufs=1) as main,
        ):
            # ---- Phase 0: input DMAs ----
            wp_sb = const.tile([128, NC4, WPACK_COLS], F32R)
            nc.sync.dma_start(wp_sb[:], wpack[:].rearrange("(c p) m -> p c m", p=128))
            gt_sb = const.tile([G, C], F32)
            nc.sync.dma_start(gt_sb[:], gselT[:])
            bp_sb = const.tile([128, 12], F32)
            nc.sync.dma_start(bp_sb[:], bpack[:])

            x_sb = main.tile([128, NC4, N], F32)
            nc.sync.dma_start(x_sb[:], x[:].rearrange("(c p) n -> p c n", p=128))

            # ---- Phase 1: GroupNorm stats -> xh (standardized x) ----
            cst = const.tile([128, NC4, 2], F32)       # per-channel sum | sumsq
            gsb = const.tile([G, 8], F32)              # group scratch
            eps_t = const.tile([G, 1], F32)
            nc.vector.memset(eps_t[:], EPS)
            gstats = const.tile([G, 2], F32)           # rs | -mean*rs
            chsc = const.tile([128, NC4, 2], F32)      # per-channel rs | bias
            xh_sb = main.tile([128, NC4, N], F32R)

            with (
                tc.tile_pool(name="sq", bufs=2) as sqp,
                tc.tile_pool(name="pst", bufs=2, space="PSUM") as pst,
            ):
                for c4 in range(NC4):
                    nc.vector.tensor_reduce(
                        cst[:, c4, 0:1], x_sb[:, c4, :], axis=AX.X, op=OP.add
                    )
                    sq = sqp.tile([128, N], F32)
                    nc.scalar.activation(
                        sq[:], x_sb[:, c4, :], AF.Square,
                        accum_out=cst[:, c4, 1:2],
                    )
                gs_ps = pst.tile([G, 2], F32)
                for c4 in range(NC4):
                    nc.tensor.matmul(
                        gs_ps[:],
                        wp_sb[:, c4, GS0:GS0 + G].bitcast(F32),
                        cst[:, c4, :],
                        start=(c4 == 0),
                        stop=(c4 == NC4 - 1),
                    )
                nc.vector.tensor_copy(gsb[:, 0:2], gs_ps[:])
                # mean = gsb[:,0], ex2 = gsb[:,1] (both already / 16384)
                nc.vector.tensor_mul(gsb[:, 2:3], gsb[:, 0:1], gsb[:, 0:1])
                nc.vector.tensor_sub(gsb[:, 3:4], gsb[:, 1:2], gsb[:, 2:3])
                nc.scalar.activation(gsb[:, 4:5], gsb[:, 3:4], AF.Ln, bias=eps_t[:])
                nc.scalar.activation(gstats[:, 0:1], gsb[:, 4:5], AF.Exp, scale=-0.5)
                nc.vector.tensor_mul(gsb[:, 6:7], gsb[:, 0:1], gstats[:, 0:1])
                nc.vector.tensor_scalar_mul(gstats[:, 1:2], gsb[:, 6:7], -1.0)
                for c4 in range(NC4):
                    cs_ps = pst.tile([128, 2], F32)
                    nc.tensor.matmul(
                        cs_ps[:],
                        gt_sb[:, c4 * 128:(c4 + 1) * 128],
                        gstats[:],
                        start=True,
                        stop=True,
                    )
                    nc.vector.tensor_copy(chsc[:, c4, :], cs_ps[:])
                for c4 in range(NC4):
                    nc.vector.tensor_scalar(
                        xh_sb[:, c4, :], x_sb[:, c4, :],
                        chsc[:, c4, 0:1], chsc[:, c4, 1:2],
                        op0=OP.mult, op1=OP.add,
                    )

            # ---- Phase 2: qkv ----
            q_sb = main.tile([128, NC4, N], F32R)
            k_sb = main.tile([128, NC4, N], F32R)
            vt_sb = main.tile([128, NM8, HEADS, HD + 1], F32R)
            ones64 = const.tile([128, NM8 * HEADS], F32)
            nc.vector.memset(ones64[:], 1.0)
            nc.vector.tensor_copy(
                vt_sb[:, :, :, HD],
                ones64[:].rearrange("p (a b) -> p a b", a=NM8),
            )

            with tc.tile_pool(name="pmm", bufs=4, space="PSUM") as pmm:
                for dst, woff, boff in ((q_sb, 0, 0), (k_sb, C, 4)):
                    for m in range(4):
                        for nh in range(2):
                            ps = pmm.tile([128, 512], F32)
                            for kc in range(NC4):
                                nc.tensor.matmul(
                                    ps[:],
                                    wp_sb[:, kc, woff + m * 128:woff + (m + 1) * 128],
                                    xh_sb[:, kc, nh * 512:(nh + 1) * 512],
                                    start=(kc == 0),
                                    stop=(kc == NC4 - 1),
                                )
                            nc.vector.tensor_scalar_add(
                                dst[:, m, nh * 512:(nh + 1) * 512], ps[:],
                                bp_sb[:, boff + m:boff + m + 1],
                            )
                for mt in range(NM8):
                    ps = pmm.tile([128, 512], F32)
                    for kc in range(NC4):
                        nc.tensor.matmul(
                            ps[:],
                            xh_sb[:, kc, mt * 128:(mt + 1) * 128],
                            wp_sb[:, kc, WV0:WV0 + C],
                            start=(kc == 0),
                            stop=(kc == NC4 - 1),
                        )
                    nc.vector.tensor_copy(
                        vt_sb[:, mt, :, 0:HD],
                        ps[:].rearrange("p (h c) -> p h c", h=HEADS),
                    )

            # ---- Phase 3: attention, head pair j = heads (2j, 2j+1) ----
            aun_ch = main.tile([128, NC4, N], F32)     # unnormalized A
            a_sb = main.tile([128, NC4, N], F32R)      # normalized A
            with (
                tc.tile_pool(name="probs", bufs=4) as probs,
                tc.tile_pool(name="spool", bufs=3) as spool,
                tc.tile_pool(name="rpool", bufs=2) as rpool,
                tc.tile_pool(name="dpool", bufs=2) as dpool,
                tc.tile_pool(name="psc", bufs=2, space="PSUM") as psc,
                tc.tile_pool(name="pav", bufs=2, space="PSUM") as pav,
            ):
                for j in range(4):
                    rows = (slice(0, HD), slice(HD, 128))
                    av_ps = [
                        pav.tile([HD + 1, N], F32, name=f"av_ps{j}_{i}", tag="av_ps")
                        for i in range(2)
                    ]
                    for kq in range(NM8):
                        for hh in range(2):
                            rs = rows[hh]
                            h = 2 * j + hh
                            sps = psc.tile([128, N], F32)
                            for nh in range(2):
                                nc.tensor.matmul(
                                    sps[:, nh * 512:(nh + 1) * 512],
                                    k_sb[rs, j, kq * 128:(kq + 1) * 128],
                                    q_sb[rs, j, nh * 512:(nh + 1) * 512],
                                    start=True,
                                    stop=True,
                                )
                            pt = probs.tile([128, N], F32R)
                            nc.scalar.activation(pt[:], sps[:], AF.Exp)
                            for nh in range(2):
                                nc.tensor.matmul(
                                    av_ps[hh][:, nh * 512:(nh + 1) * 512],
                                    vt_sb[:, kq, h, :],
                                    pt[:, nh * 512:(nh + 1) * 512],
                                    start=(kq == 0),
                                    stop=(kq == NM8 - 1),
                                )
                    for hh in range(2):
                        h = 2 * j + hh
                        s_t = spool.tile([HD + 1, N], F32)
                        nc.vector.tensor_copy(s_t[:], av_ps[hh][:])
                        nc.sync.dma_start(dall[h:h + 1, :], s_t[HD:HD + 1, :])
                        nc.sync.dma_start(aun_ch[rows[hh], j, :], s_t[0:HD, :])
                    # reciprocal of the pair's denominators, then broadcast
                    dd = dpool.tile([128, 2, 8], F32)
                    nc.sync.dma_start(
                        dd[:], dall[2 * j:2 * j + 2, :].rearrange(
                            "h (p f) -> p h f", f=8
                        )
                    )
                    rr = dpool.tile([128, 2, 8], F32)
                    nc.vector.reciprocal(rr[:], dd[:])
                    nc.sync.dma_start(
                        rdram[2 * j:2 * j + 2, :].rearrange("h (p f) -> p h f", f=8),
                        rr[:],
                    )
                    rt = rpool.tile([128, N], F32)
                    rsrc = rdram[2 * j:2 * j + 2, :]
                    nc.sync.dma_start(
                        rt[:],
                        bass.AP(
                            tensor=rsrc.tensor,
                            offset=rsrc.offset,
                            ap=[[N, 2], [0, HD], [1, N]],
                        ),
                    )
                    nc.vector.tensor_mul(a_sb[:, j, :], aun_ch[:, j, :], rt[:])

            # ---- Phase 4: proj + bias(+v-bias fold) + residual ----
            with (
                tc.tile_pool(name="ppj", bufs=4, space="PSUM") as ppj,
                tc.tile_pool(name="ypool", bufs=4) as ypool,
            ):
                yv = y[:].rearrange("(m p) n -> m p n", p=128)
                for m in range(4):
                    for nh in range(2):
                        ps = ppj.tile([128, 512], F32)
                        for kc in range(NC4):
                            nc.tensor.matmul(
                                ps[:],
                                wp_sb[:, kc, WP0 + m * 128:WP0 + (m + 1) * 128],
                                a_sb[:, kc, nh * 512:(nh + 1) * 512],
                                start=(kc == 0),
                                stop=(kc == NC4 - 1),
                            )
                        yt = ypool.tile([128, 512], F32)
                        nc.scalar.activation(
                            yt[:], ps[:], AF.Identity,
                            bias=bp_sb[:, 8 + m:9 + m],
                        )
                        nc.vector.tensor_add(
                            yt[:], yt[:], x_sb[:, m, nh * 512:(nh + 1) * 512]
                        )
                        nc.sync.dma_start(yv[m, :, nh * 512:(nh + 1) * 512], yt[:])

    nc.compile()
    return nc


_NC_CACHE = None


def _get_nc():
    global _NC_CACHE
    if _NC_CACHE is None:
        _NC_CACHE = build_nc()
    return _NC_CACHE


def _prep_host(norm_w, norm_b, qkv_w, qkv_b, proj_w, proj_b):
    g = norm_w.astype(np.float32)
    b = norm_b.astype(np.float32)
    Wq, Wk, Wv = qkv_w[0:C], qkv_w[C:2 * C], qkv_w[2 * C:3 * C]
    bq, bk, bv = qkv_b[0:C], qkv_b[C:2 * C], qkv_b[2 * C:3 * C]
    scale = np.float32(1.0 / np.sqrt(HD))

    WqT = (scale * (Wq * g[None, :])).T
    WkT = (Wk * g[None, :]).T
    WvT = (Wv * g[None, :]).T
    bq_eff = scale * (Wq @ b + bq)
    bk_eff = Wk @ b + bk
    pb_eff = proj_w @ (Wv @ b + bv) + proj_b

    cidx = np.arange(C)
    gsel = np.zeros((C, G), np.float32)
    gsel[cidx, cidx // GSZ] = np.float32(1.0 / (GSZ * N))
    gselT = np.zeros((G, C), np.float32)
    gselT[cidx // GSZ, cidx] = 1.0

    wpack = np.concatenate(
        [WqT, WkT, WvT, proj_w.T, gsel], axis=1
    ).astype(np.float32)
    assert wpack.shape == (C, WPACK_COLS)

    bpack = np.stack(
        [bq_eff.reshape(4, 128), bk_eff.reshape(4, 128),
         pb_eff.reshape(4, 128)], axis=0,
    ).reshape(12, 128).T.astype(np.float32)
    return np.ascontiguousarray(wpack), gselT, np.ascontiguousarray(bpack)


def kernel(x, norm_w, norm_b, qkv_w, qkv_b, proj_w, proj_b):
    b_sz, c, h, w = x.shape
    assert (b_sz, c, h * w) == (8, C, N)
    wpack, gselT, bpack = _prep_host(norm_w, norm_b, qkv_w, qkv_b, proj_w, proj_b)
    xf = np.ascontiguousarray(x.reshape(b_sz, C, N).astype(np.float32))

    nc = _get_nc()
    in_maps = [
        {"x": xf[i], "wpack": wpack, "gselT": gselT, "bpack": bpack}
        for i in range(b_sz)
    ]
    res = run_bass_kernel_spmd(nc, in_maps, core_ids=list(range(b_sz)))
    out = np.stack([r["y"] for r in res.results], axis=0)
    return out.reshape(b_sz, C, h, w)


# revision 20
# speedup vs baseline: 1.0105x; 1.0105x over previous
"""Trainium2 Bass kernel for nn_AttentionBlock (GroupNorm + MHA + proj + residual).

Problem: x[8, 512, 32, 32] fp32; GroupNorm(32 groups) -> qkv (1x1 conv) ->
8-head attention over 1024 spatial positions -> proj -> residual.

Sharding: data-parallel over batch. 8 batch elements -> 8 NeuronCores,
one SPMD NEFF. No collectives.

Per-core design (x as [c=512, n=1024]):
  - GroupNorm: per-channel sum/sumsq (DVE reduce + ACT square-accum), group
    reduce via a tiny matmul with a host-built selection matrix (1/16384
    entries), rsqrt via exp(-0.5*ln(var+eps)), per-channel affine via
    tensor_scalar. gamma/beta are folded into qkv weights/biases on host.
  - qkv: q,k as [512, 1024] (heads on partitions); v computed TRANSPOSED
    (v^T [1024, 512]) directly by swapping matmul operands, augmented with a
    ones column per head for the softmax denominator.
  - attention per head: scores^T = k_h^T q_h (K=64, head pairs at partition
    bases 0/64 row-pack on the PE); probs = exp(scores^T) via ACT (the 1/8
    scale is folded into Wq on host; no max-subtraction needed since
    |scores| < ~10); av+denominator in one matmul with v^T|ones as the
    stationary operand; normalization by broadcast reciprocal via small
    DRAM round-trips (DMA engines) and one tensor_tensor multiply.
  - proj: K=128 matmuls on the assembled, normalized A [512, 1024];
    v-bias and proj-bias folded into one per-channel bias on host;
    residual added during eviction.
  - Matmuls run in float32r (full-rate fp32 on the PE); operands are
    "rounded" either by declaring DRAM inputs float32r or by DVE/ACT
    eviction into float32r tiles.
"""
import sys

sys.path.insert(0, "/opt/trn_rl_repo")

import numpy as np

import concourse.bass as bass
import concourse.bacc as bacc
import concourse.tile as tile
from concourse import mybir
from concourse.bass_utils import run_bass_kernel_spmd

F32 = mybir.dt.float32
F32R = mybir.dt.float32r
BF16 = mybir.dt.bfloat16
AX = mybir.AxisListType
OP = mybir.AluOpType
AF = mybir.ActivationFunctionType

C = 512          # channels
N = 1024         # spatial positions (32*32)
HEADS = 8
HD = 64          # head dim
G = 32           # groups
GSZ = 16         # channels per group
EPS = 1e-6
NC4 = 4          # channel chunks of 128
NM8 = 8          # spatial chunks of 128
GS0 = 1024       # wpack col offsets: wqkT | gsel
WPACK_COLS = 1056
WBF_COLS = 1024  # wbf: wvT | ptT (bf16)


def build_nc():
    nc = bacc.Bacc(None)
    x = nc.declare_dram_parameter("x", [C, N], F32, isOutput=False)
    wpack = nc.declare_dram_parameter("wpack", [C, WPACK_COLS], F32R, isOutput=False)
    wbf = nc.declare_dram_parameter("wbf", [C, WBF_COLS], BF16, isOutput=False)
    gselT = nc.declare_dram_parameter("gselT", [G, C], F32, isOutput=False)
    bpack = nc.declare_dram_parameter("bpack", [128, 12], F32, isOutput=False)
    y = nc.declare_dram_parameter("y", [C, N], F32, isOutput=True)

    dall = nc.dram_tensor("dall", [HEADS, N], F32)
    rdram = nc.dram_tensor("rdram", [HEADS, N], F32)

    with tile.TileContext(nc) as tc:
        with (
            tc.tile_pool(name="const", bufs=1) as const,
            tc.tile_pool(name="main", bufs=1) as main,
        ):
            # ---- Phase 0: input DMAs ----
            wp_sb = const.tile([128, NC4, WPACK_COLS], F32R)
            nc.sync.dma_start(wp_sb[:], wpack[:].rearrange("(c p) m -> p c m", p=128))
            wbf_sb = const.tile([128, NC4, WBF_COLS], BF16)
            nc.sync.dma_start(wbf_sb[:], wbf[:].rearrange("(c p) m -> p c m", p=128))
            gt_sb = const.tile([G, C], F32)
            nc.sync.dma_start(gt_sb[:], gselT[:])
            bp_sb = const.tile([128, 12], F32)
            nc.sync.dma_start(bp_sb[:], bpack[:])

            x_sb = main.tile([128, NC4, N], F32)
            nc.sync.dma_start(x_sb[:], x[:].rearrange("(c p) n -> p c n", p=128))

            # ---- Phase 1: GroupNorm stats -> xh (standardized x) ----
            cst = const.tile([128, NC4, 2], F32)       # per-channel sum | sumsq
            gsb = const.tile([G, 8], F32)              # group scratch
            eps_t = const.tile([G, 1], F32)
            nc.vector.memset(eps_t[:], EPS)
            gstats = const.tile([G, 2], F32)           # rs | -mean*rs
            chsc = const.tile([128, NC4, 2], F32)      # per-channel rs | bias
            xh_sb = main.tile([128, NC4, N], F32R)
            xhbf_sb = main.tile([128, NC4, N], BF16)   # for the v^T matmul

            with (
                tc.tile_pool(name="sq", bufs=2) as sqp,
                tc.tile_pool(name="pst", bufs=2, space="PSUM") as pst,
            ):
                for c4 in range(NC4):
                    nc.vector.tensor_reduce(
                        cst[:, c4, 0:1], x_sb[:, c4, :], axis=AX.X, op=OP.add
                    )
                    sq = sqp.tile([128, N], F32)
                    nc.scalar.activation(
                        sq[:], x_sb[:, c4, :], AF.Square,
                        accum_out=cst[:, c4, 1:2],
                    )
                gs_ps = pst.tile([G, 2], F32)
                for c4 in range(NC4):
                    nc.tensor.matmul(
                        gs_ps[:],
                        wp_sb[:, c4, GS0:GS0 + G].bitcast(F32),
                        cst[:, c4, :],
                        start=(c4 == 0),
                        stop=(c4 == NC4 - 1),
                    )
                nc.vector.tensor_copy(gsb[:, 0:2], gs_ps[:])
                # mean = gsb[:,0], ex2 = gsb[:,1] (both already / 16384)
                nc.vector.tensor_mul(gsb[:, 2:3], gsb[:, 0:1], gsb[:, 0:1])
                nc.vector.tensor_sub(gsb[:, 3:4], gsb[:, 1:2], gsb[:, 2:3])
                nc.scalar.activation(gsb[:, 4:5], gsb[:, 3:4], AF.Ln, bias=eps_t[:])
                nc.scalar.activation(gstats[:, 0:1], gsb[:, 4:5], AF.Exp, scale=-0.5)
                nc.vector.tensor_mul(gsb[:, 6:7], gsb[:, 0:1], gstats[:, 0:1])
                nc.vector.tensor_scalar_mul(gstats[:, 1:2], gsb[:, 6:7], -1.0)
                for c4 in range(NC4):
                    cs_ps = pst.tile([128, 2], F32)
                    nc.tensor.matmul(
                        cs_ps[:],
                        gt_sb[:, c4 * 128:(c4 + 1) * 128],
                        gstats[:],
                        start=True,
                        stop=True,
                    )
                    nc.vector.tensor_copy(chsc[:, c4, :], cs_ps[:])
                for c4 in range(NC4):
                    nc.vector.tensor_scalar(
                        xh_sb[:, c4, :], x_sb[:, c4, :],
                        chsc[:, c4, 0:1], chsc[:, c4, 1:2],
                        op0=OP.mult, op1=OP.add,
                    )
                    nc.vector.tensor_copy(xhbf_sb[:, c4, :], xh_sb[:, c4, :])

            # ---- Phase 2: qkv ----
            q_sb = main.tile([128, NC4, N], F32R)
            k_sb = main.tile([128, NC4, N], F32R)
            vt_sb = main.tile([128, NM8, HEADS, HD + 1], BF16)
            ones64 = const.tile([128, NM8 * HEADS], BF16)
            nc.vector.memset(ones64[:], 1.0)
            nc.vector.tensor_copy(
                vt_sb[:, :, :, HD],
                ones64[:].rearrange("p (a b) -> p a b", a=NM8),
            )

            with tc.tile_pool(name="pmm", bufs=4, space="PSUM") as pmm:
                for dst, woff, boff in ((q_sb, 0, 0), (k_sb, C, 4)):
                    for m in range(4):
                        for nh in range(2):
                            ps = pmm.tile([128, 512], F32)
                            for kc in range(NC4):
                                nc.tensor.matmul(
                                    ps[:],
                                    wp_sb[:, kc, woff + m * 128:woff + (m + 1) * 128],
                                    xh_sb[:, kc, nh * 512:(nh + 1) * 512],
                                    start=(kc == 0),
                                    stop=(kc == NC4 - 1),
                                )
                            nc.vector.tensor_scalar_add(
                                dst[:, m, nh * 512:(nh + 1) * 512], ps[:],
                                bp_sb[:, boff + m:boff + m + 1],
                            )
                for mt in range(NM8):
                    ps = pmm.tile([128, 512], F32)
                    for kc in range(NC4):
                        nc.tensor.matmul(
                            ps[:],
                            xhbf_sb[:, kc, mt * 128:(mt + 1) * 128],
                            wbf_sb[:, kc, 0:C],
                            start=(kc == 0),
                            stop=(kc == NC4 - 1),
                        )
                    nc.vector.tensor_copy(
                        vt_sb[:, mt, :, 0:HD],
                        ps[:].rearrange("p (h c) -> p h c", h=HEADS),
                    )

            # ---- Phase 3: attention, head pair j = heads (2j, 2j+1) ----
            aun_ch = main.tile([128, NC4, N], F32)     # unnormalized A
            a_sb = main.tile([128, NC4, N], BF16)      # normalized A
            with (
                tc.tile_pool(name="probs", bufs=4) as probs,
                tc.tile_pool(name="spool", bufs=3) as spool,
                tc.tile_pool(name="rpool", bufs=2) as rpool,
                tc.tile_pool(name="dpool", bufs=2) as dpool,
                tc.tile_pool(name="psc", bufs=2, space="PSUM") as psc,
                tc.tile_pool(name="pav", bufs=2, space="PSUM") as pav,
            ):
                for j in range(4):
                    rows = (slice(0, HD), slice(HD, 128))
                    av_ps = [
                        pav.tile([HD + 1, N], F32, name=f"av_ps{j}_{i}", tag="av_ps")
                        for i in range(2)
                    ]
                    for kq in range(NM8):
                        for hh in range(2):
                            rs = rows[hh]
                            h = 2 * j + hh
                            sps = psc.tile([128, N], F32)
                            for nh in range(2):
                                nc.tensor.matmul(
                                    sps[:, nh * 512:(nh + 1) * 512],
                                    k_sb[rs, j, kq * 128:(kq + 1) * 128],
                                    q_sb[rs, j, nh * 512:(nh + 1) * 512],
                                    start=True,
                                    stop=True,
                                )
                            pt = probs.tile([128, N], BF16)
                            nc.scalar.activation(pt[:], sps[:], AF.Exp)
                            for nh in range(2):
                                nc.tensor.matmul(
                                    av_ps[hh][:, nh * 512:(nh + 1) * 512],
                                    vt_sb[:, kq, h, :],
                                    pt[:, nh * 512:(nh + 1) * 512],
                                    start=(kq == 0),
                                    stop=(kq == NM8 - 1),
                                )
                    for hh in range(2):
                        h = 2 * j + hh
                        s_t = spool.tile([HD + 1, N], F32)
                        nc.vector.tensor_copy(s_t[:], av_ps[hh][:])
                        nc.sync.dma_start(dall[h:h + 1, :], s_t[HD:HD + 1, :])
                        nc.sync.dma_start(aun_ch[rows[hh], j, :], s_t[0:HD, :])
                    # reciprocal of the pair's denominators, then broadcast
                    dd = dpool.tile([128, 2, 8], F32)
                    nc.sync.dma_start(
                        dd[:], dall[2 * j:2 * j + 2, :].rearrange(
                            "h (p f) -> p h f", f=8
                        )
                    )
                    rr = dpool.tile([128, 2, 8], F32)
                    nc.vector.reciprocal(rr[:], dd[:])
                    nc.sync.dma_start(
                        rdram[2 * j:2 * j + 2, :].rearrange("h (p f) -> p h f", f=8),
                        rr[:],
                    )
                    rt = rpool.tile([128, N], F32)
                    rsrc = rdram[2 * j:2 * j + 2, :]
                    nc.sync.dma_start(
                        rt[:],
                        bass.AP(
                            tensor=rsrc.tensor,
                            offset=rsrc.offset,
                            ap=[[N, 2], [0, HD], [1, N]],
                        ),
                    )
                    nc.vector.tensor_mul(a_sb[:, j, :], aun_ch[:, j, :], rt[:])

            # ---- Phase 4: proj + bias(+v-bias fold) + residual ----
            with (
                tc.tile_pool(name="ppj", bufs=4, space="PSUM") as ppj,
                tc.tile_pool(name="ypool", bufs=4) as ypool,
            ):
                yv = y[:].rearrange("(m p) n -> m p n", p=128)
                for m in range(4):
                    for nh in range(2):
                        ps = ppj.tile([128, 512], F32)
                        for kc in range(NC4):
                            nc.tensor.matmul(
                                ps[:],
                                wbf_sb[:, kc, C + m * 128:C + (m + 1) * 128],
                                a_sb[:, kc, nh * 512:(nh + 1) * 512],
                                start=(kc == 0),
                                stop=(kc == NC4 - 1),
                            )
                        yt = ypool.tile([128, 512], F32)
                        nc.scalar.activation(
                            yt[:], ps[:], AF.Identity,
                            bias=bp_sb[:, 8 + m:9 + m],
                        )
                        nc.vector.tensor_add(
                            yt[:], yt[:], x_sb[:, m, nh * 512:(nh + 1) * 512]
                        )
                        nc.sync.dma_start(yv[m, :, nh * 512:(nh + 1) * 512], yt[:])

    nc.compile()
    return nc


_NC_CACHE = None


def _get_nc():
    global _NC_CACHE
    if _NC_CACHE is None:
        _NC_CACHE = build_nc()
    return _NC_CACHE


def _prep_host(norm_w, norm_b, qkv_w, qkv_b, proj_w, proj_b):
    g = norm_w.astype(np.float32)
    b = norm_b.astype(np.float32)
    Wq, Wk, Wv = qkv_w[0:C], qkv_w[C:2 * C], qkv_w[2 * C:3 * C]
    bq, bk, bv = qkv_b[0:C], qkv_b[C:2 * C], qkv_b[2 * C:3 * C]
    scale = np.float32(1.0 / np.sqrt(HD))

    WqT = (scale * (Wq * g[None, :])).T
    WkT = (Wk * g[None, :]).T
    WvT = (Wv * g[None, :]).T
    bq_eff = scale * (Wq @ b + bq)
    bk_eff = Wk @ b + bk
    pb_eff = proj_w @ (Wv @ b + bv) + proj_b

    cidx = np.arange(C)
    gsel = np.zeros((C, G), np.float32)
    gsel[cidx, cidx // GSZ] = np.float32(1.0 / (GSZ * N))
    gselT = np.zeros((G, C), np.float32)
    gselT[cidx // GSZ, cidx] = 1.0

    wpack = np.concatenate([WqT, WkT, gsel], axis=1).astype(np.float32)
    assert wpack.shape == (C, WPACK_COLS)
    import ml_dtypes

    wbf = np.concatenate([WvT, proj_w.T], axis=1).astype(ml_dtypes.bfloat16)
    assert wbf.shape == (C, WBF_COLS)

    bpack = np.stack(
        [bq_eff.reshape(4, 128), bk_eff.reshape(4, 128),
         pb_eff.reshape(4, 128)], axis=0,
    ).reshape(12, 128).T.astype(np.float32)
    return (np.ascontiguousarray(wpack), np.ascontiguousarray(wbf),
            gselT, np.ascontiguousarray(bpack))


def kernel(x, norm_w, norm_b, qkv_w, qkv_b, proj_w, proj_b):
    b_sz, c, h, w = x.shape
    assert (b_sz, c, h * w) == (8, C, N)
    wpack, wbf, gselT, bpack = _prep_host(
        norm_w, norm_b, qkv_w, qkv_b, proj_w, proj_b
    )
    xf = np.ascontiguousarray(x.reshape(b_sz, C, N).astype(np.float32))

    nc = _get_nc()
    in_maps = [
        {"x": xf[i], "wpack": wpack, "wbf": wbf, "gselT": gselT, "bpack": bpack}
        for i in range(b_sz)
    ]
    res = run_bass_kernel_spmd(nc, in_maps, core_ids=list(range(b_sz)))
    out = np.stack([r["y"] for r in res.results], axis=0)
    return out.reshape(b_sz, C, h, w)


# revision 29
# speedup vs baseline: 1.0676x; 1.0565x over previous
"""Trainium2 Bass kernel for nn_AttentionBlock (GroupNorm + MHA + proj + residual).

Problem: x[8, 512, 32, 32] fp32; GroupNorm(32 groups) -> qkv (1x1 conv) ->
8-head attention over 1024 spatial positions -> proj -> residual.

Sharding: data-parallel over batch. 8 batch elements -> 8 NeuronCores,
one SPMD NEFF. No collectives.

Per-core design (x as [c=512, n=1024]):
  - GroupNorm: per-channel sum/sumsq (DVE reduce + ACT square-accum), group
    reduce via a tiny matmul with a host-built selection matrix (1/16384
    entries), rsqrt via exp(-0.5*ln(var+eps)), per-channel affine via
    tensor_scalar. gamma/beta are folded into qkv weights/biases on host.
  - qkv: q,k as [512, 1024] (heads on partitions); v computed TRANSPOSED
    (v^T [1024, 512]) directly by swapping matmul operands, augmented with a
    ones column per head for the softmax denominator.
  - attention per head: scores^T = k_h^T q_h (K=64, head pairs at partition
    bases 0/64 row-pack on the PE); probs = exp(scores^T) via ACT (the 1/8
    scale is folded into Wq on host; no max-subtraction needed since
    |scores| < ~10); av+denominator in one matmul with v^T|ones as the
    stationary operand; normalization by broadcast reciprocal via small
    DRAM round-trips (DMA engines) and one tensor_tensor multiply.
  - proj: K=128 matmuls on the assembled, normalized A [512, 1024];
    v-bias and proj-bias folded into one per-channel bias on host;
    residual added during eviction.
  - Matmuls run in float32r (full-rate fp32 on the PE); operands are
    "rounded" either by declaring DRAM inputs float32r or by DVE/ACT
    eviction into float32r tiles.
"""
import sys

sys.path.insert(0, "/opt/trn_rl_repo")

import numpy as np

import concourse.bass as bass
import concourse.bacc as bacc
import concourse.tile as tile
from concourse import mybir
from concourse.bass_utils import run_bass_kernel_spmd

F32 = mybir.dt.float32
F32R = mybir.dt.float32r
AX = mybir.AxisListType
OP = mybir.AluOpType
AF = mybir.ActivationFunctionType

C = 512          # channels
N = 1024         # spatial positions (32*32)
HEADS = 8
HD = 64          # head dim
G = 32           # groups
GSZ = 16         # channels per group
EPS = 1e-6
NC4 = 4          # channel chunks of 128
NM8 = 8          # spatial chunks of 128
WQK = 1024       # wpack col offsets
WV0 = 1024
WP0 = 1536
WPACK_COLS = 2048


def build_nc():
    nc = bacc.Bacc(None)
    x = nc.declare_dram_parameter("x", [C, N], F32, isOutput=False)
    wpack = nc.declare_dram_parameter("wpack", [C, WPACK_COLS], F32R, isOutput=False)
    gsel = nc.declare_dram_parameter("gsel", [C, G], F32, isOutput=False)
    gselT = nc.declare_dram_parameter("gselT", [G, C], F32, isOutput=False)
    bpack = nc.declare_dram_parameter("bpack", [128, 12], F32, isOutput=False)
    y = nc.declare_dram_parameter("y", [C, N], F32, isOutput=True)

    rdram = nc.dram_tensor("rdram", [HEADS, N], F32)

    with tile.TileContext(nc) as tc:
        with (
            tc.tile_pool(name="const", bufs=1) as const,
            tc.tile_pool(name="main", bufs=1) as main,
        ):
            # ---- Phase 0: input DMAs (x first — everything waits on it) ----
            x_sb = main.tile([128, NC4, N], F32)
            xv = x[:].rearrange("(c p) n -> p c n", p=128)
            for c4 in range(NC4):
                nc.sync.dma_start(x_sb[:, c4, :], xv[:, c4, :])
            gs_sb = const.tile([128, NC4, G], F32)
            nc.sync.dma_start(gs_sb[:], gsel[:].rearrange("(c p) g -> p c g", p=128))
            gt_sb = const.tile([G, C], F32)
            nc.sync.dma_start(gt_sb[:], gselT[:])
            bp_sb = const.tile([128, 12], F32)
            nc.sync.dma_start(bp_sb[:], bpack[:])
            wp_sb = const.tile([128, NC4, WPACK_COLS], F32R)
            wpv = wpack[:].rearrange("(c p) m -> p c m", p=128)
            for c4 in range(NC4):
                nc.sync.dma_start(wp_sb[:, c4, :], wpv[:, c4, :])

            # ---- Phase 1: GroupNorm stats -> xh (standardized x) ----
            cst = const.tile([128, NC4, 2], F32)       # per-channel sum | sumsq
            gsb = const.tile([G, 8], F32)              # group scratch
            eps_t = const.tile([G, 1], F32)
            nc.vector.memset(eps_t[:], EPS)
            gstats = const.tile([G, 2], F32)           # rs | -mean*rs
            chsc = const.tile([128, NC4, 2], F32)      # per-channel rs | bias
            xh_sb = main.tile([128, NC4, N], F32R)

            # prefire the Ln/Exp ACT table load while the x DMA runs
            nc.scalar.activation(gsb[:, 4:5], eps_t[:], AF.Ln, bias=eps_t[:])
            nc.scalar.activation(gsb[:, 4:5], gsb[:, 4:5], AF.Exp)

            with (
                tc.tile_pool(name="sq", bufs=2) as sqp,
                tc.tile_pool(name="pst", bufs=2, space="PSUM") as pst,
            ):
                for c4 in range(NC4):
                    nc.vector.tensor_reduce(
                        cst[:, c4, 0:1], x_sb[:, c4, :], axis=AX.X, op=OP.add
                    )
                    sq = sqp.tile([128, N], F32)
                    nc.scalar.activation(
                        sq[:], x_sb[:, c4, :], AF.Square,
                        accum_out=cst[:, c4, 1:2],
                    )
                gs_ps = pst.tile([G, 2], F32)
                for c4 in range(NC4):
                    nc.tensor.matmul(
                        gs_ps[:],
                        gs_sb[:, c4, :],
                        cst[:, c4, :],
                        start=(c4 == 0),
                        stop=(c4 == NC4 - 1),
                    )
                nc.vector.tensor_copy(gsb[:, 0:2], gs_ps[:])
                # mean = gsb[:,0], ex2 = gsb[:,1] (both already / 16384)
                nc.vector.tensor_mul(gsb[:, 2:3], gsb[:, 0:1], gsb[:, 0:1])
                nc.vector.tensor_sub(gsb[:, 3:4], gsb[:, 1:2], gsb[:, 2:3])
                nc.scalar.activation(gsb[:, 4:5], gsb[:, 3:4], AF.Ln, bias=eps_t[:])
                nc.scalar.activation(gstats[:, 0:1], gsb[:, 4:5], AF.Exp, scale=-0.5)
                nc.vector.tensor_mul(gsb[:, 6:7], gsb[:, 0:1], gstats[:, 0:1])
                nc.vector.tensor_scalar_mul(gstats[:, 1:2], gsb[:, 6:7], -1.0)
                for c4 in range(NC4):
                    cs_ps = pst.tile([128, 2], F32)
                    nc.tensor.matmul(
                        cs_ps[:],
                        gt_sb[:, c4 * 128:(c4 + 1) * 128],
                        gstats[:],
                        start=True,
                        stop=True,
                    )
                    nc.vector.tensor_copy(chsc[:, c4, :], cs_ps[:])
                for c4 in range(NC4):
                    nc.vector.tensor_scalar(
                        xh_sb[:, c4, :], x_sb[:, c4, :],
                        chsc[:, c4, 0:1], chsc[:, c4, 1:2],
                        op0=OP.mult, op1=OP.add,
                    )

            # ---- Phase 2: qkv ----
            q_sb = main.tile([128, NC4, N], F32R)
            k_sb = main.tile([128, NC4, N], F32R)
            vt_sb = main.tile([128, NM8, HEADS, HD + 1], F32R)
            ones64 = const.tile([128, NM8 * HEADS], F32)
            nc.vector.memset(ones64[:], 1.0)
            nc.vector.tensor_copy(
                vt_sb[:, :, :, HD],
                ones64[:].rearrange("p (a b) -> p a b", a=NM8),
            )

            with tc.tile_pool(name="pmm", bufs=4, space="PSUM") as pmm:
                for dst, woff, boff in ((q_sb, 0, 0), (k_sb, C, 4)):
                    for m in range(4):
                        for nh in range(2):
                            ps = pmm.tile([128, 512], F32)
                            for kc in range(NC4):
                                nc.tensor.matmul(
                                    ps[:],
                                    wp_sb[:, kc, woff + m * 128:woff + (m + 1) * 128],
                                    xh_sb[:, kc, nh * 512:(nh + 1) * 512],
                                    start=(kc == 0),
                                    stop=(kc == NC4 - 1),
                                )
                            nc.vector.tensor_scalar_add(
                                dst[:, m, nh * 512:(nh + 1) * 512], ps[:],
                                bp_sb[:, boff + m:boff + m + 1],
                            )
                for mt in range(NM8):
                    ps = pmm.tile([128, 512], F32)
                    for kc in range(NC4):
                        nc.tensor.matmul(
                            ps[:],
                            xh_sb[:, kc, mt * 128:(mt + 1) * 128],
                            wp_sb[:, kc, WV0:WV0 + C],
                            start=(kc == 0),
                            stop=(kc == NC4 - 1),
                        )
                    nc.vector.tensor_copy(
                        vt_sb[:, mt, :, 0:HD],
                        ps[:].rearrange("p (h c) -> p h c", h=HEADS),
                    )

            # ---- Phase 3: attention, head pair j = heads (2j, 2j+1) ----
            aun_ch = main.tile([128, NC4, N], F32)     # unnormalized A
            a_sb = main.tile([128, NC4, N], F32R)      # normalized A
            with (
                tc.tile_pool(name="probs", bufs=4) as probs,
                tc.tile_pool(name="spool", bufs=3) as spool,
                tc.tile_pool(name="rpool", bufs=2) as rpool,
                tc.tile_pool(name="dpool", bufs=2) as dpool,
                tc.tile_pool(name="psc", bufs=2, space="PSUM") as psc,
                tc.tile_pool(name="pav", bufs=2, space="PSUM") as pav,
            ):
                for j in range(4):
                    rows = (slice(0, HD), slice(HD, 128))
                    av_ps = [
                        pav.tile([HD + 1, N], F32, name=f"av_ps{j}_{i}", tag="av_ps")
                        for i in range(2)
                    ]
                    for kq in range(NM8):
                        for hh in range(2):
                            rs = rows[hh]
                            h = 2 * j + hh
                            sps = psc.tile([128, N], F32)
                            for nh in range(2):
                                nc.tensor.matmul(
                                    sps[:, nh * 512:(nh + 1) * 512],
                                    k_sb[rs, j, kq * 128:(kq + 1) * 128],
                                    q_sb[rs, j, nh * 512:(nh + 1) * 512],
                                    start=True,
                                    stop=True,
                                )
                            pt = probs.tile([128, N], F32R)
                            nc.scalar.activation(pt[:], sps[:], AF.Exp)
                            for nh in range(2):
                                nc.tensor.matmul(
                                    av_ps[hh][:, nh * 512:(nh + 1) * 512],
                                    vt_sb[:, kq, h, :],
                                    pt[:, nh * 512:(nh + 1) * 512],
                                    start=(kq == 0),
                                    stop=(kq == NM8 - 1),
                                )
                    dd = dpool.tile([128, 2, 8], F32)
                    for hh in range(2):
                        h = 2 * j + hh
                        s_t = spool.tile([HD + 1, N], F32)
                        nc.vector.tensor_copy(s_t[:], av_ps[hh][:])
                        # denominator row -> [128, 8] layout (dd[p,hh,f] = d[8p+f])
                        nc.sync.dma_start(dd[:, hh, :], s_t[HD:HD + 1, :])
                        nc.sync.dma_start(aun_ch[rows[hh], j, :], s_t[0:HD, :])
                    # reciprocal of the pair's denominators, then broadcast
                    rr = dpool.tile([128, 2, 8], F32)
                    nc.vector.reciprocal(rr[:], dd[:])
                    nc.sync.dma_start(
                        rdram[2 * j:2 * j + 2, :].rearrange("h (p f) -> p h f", f=8),
                        rr[:],
                    )
                    rt = rpool.tile([128, N], F32)
                    rsrc = rdram[2 * j:2 * j + 2, :]
                    nc.sync.dma_start(
                        rt[:],
                        bass.AP(
                            tensor=rsrc.tensor,
                            offset=rsrc.offset,
                            ap=[[N, 2], [0, HD], [1, N]],
                        ),
                    )
                    nc.vector.tensor_mul(a_sb[:, j, :], aun_ch[:, j, :], rt[:])

            # ---- Phase 4: proj + bias(+v-bias fold) + residual ----
            with (
                tc.tile_pool(name="ppj", bufs=4, space="PSUM") as ppj,
                tc.tile_pool(name="ypool", bufs=4) as ypool,
            ):
                yv = y[:].rearrange("(m p) n -> m p n", p=128)
                for m in range(4):
                    for nh in range(2):
                        ps = ppj.tile([128, 512], F32)
                        for kc in range(NC4):
                            nc.tensor.matmul(
                                ps[:],
                                wp_sb[:, kc, WP0 + m * 128:WP0 + (m + 1) * 128],
                                a_sb[:, kc, nh * 512:(nh + 1) * 512],
                                start=(kc == 0),
                                stop=(kc == NC4 - 1),
                            )
                        yt = ypool.tile([128, 512], F32)
                        nc.scalar.activation(
                            yt[:], ps[:], AF.Identity,
                            bias=bp_sb[:, 8 + m:9 + m],
                        )
                        nc.vector.tensor_add(
                            yt[:], yt[:], x_sb[:, m, nh * 512:(nh + 1) * 512]
                        )
                        nc.sync.dma_start(yv[m, :, nh * 512:(nh + 1) * 512], yt[:])

    nc.compile()
    return nc


_NC_CACHE = None


def _get_nc():
    global _NC_CACHE
    if _NC_CACHE is None:
        _NC_CACHE = build_nc()
    return _NC_CACHE


def _prep_host(norm_w, norm_b, qkv_w, qkv_b, proj_w, proj_b):
    g = norm_w.astype(np.float32)
    b = norm_b.astype(np.float32)
    Wq, Wk, Wv = qkv_w[0:C], qkv_w[C:2 * C], qkv_w[2 * C:3 * C]
    bq, bk, bv = qkv_b[0:C], qkv_b[C:2 * C], qkv_b[2 * C:3 * C]
    scale = np.float32(1.0 / np.sqrt(HD))

    WqT = (scale * (Wq * g[None, :])).T
    WkT = (Wk * g[None, :]).T
    WvT = (Wv * g[None, :]).T
    bq_eff = scale * (Wq @ b + bq)
    bk_eff = Wk @ b + bk
    pb_eff = proj_w @ (Wv @ b + bv) + proj_b

    cidx = np.arange(C)
    gsel = np.zeros((C, G), np.float32)
    gsel[cidx, cidx // GSZ] = np.float32(1.0 / (GSZ * N))
    gselT = np.zeros((G, C), np.float32)
    gselT[cidx // GSZ, cidx] = 1.0

    wpack = np.concatenate([WqT, WkT, WvT, proj_w.T], axis=1).astype(np.float32)
    assert wpack.shape == (C, WPACK_COLS)

    bpack = np.stack(
        [bq_eff.reshape(4, 128), bk_eff.reshape(4, 128),
         pb_eff.reshape(4, 128)], axis=0,
    ).reshape(12, 128).T.astype(np.float32)
    return (np.ascontiguousarray(wpack), np.ascontiguousarray(gsel), gselT,
            np.ascontiguousarray(bpack))


def make_in_maps(x, norm_w, norm_b, qkv_w, qkv_b, proj_w, proj_b):
    b_sz = x.shape[0]
    wpack, gsel, gselT, bpack = _prep_host(
        norm_w, norm_b, qkv_w, qkv_b, proj_w, proj_b
    )
    xf = np.ascontiguousarray(x.reshape(b_sz, C, N).astype(np.float32))
    return [
        {"x": xf[i], "wpack": wpack, "gsel": gsel, "gselT": gselT,
         "bpack": bpack}
        for i in range(b_sz)
    ]


def kernel(x, norm_w, norm_b, qkv_w, qkv_b, proj_w, proj_b):
    b_sz, c, h, w = x.shape
    assert (b_sz, c, h * w) == (8, C, N)
    nc = _get_nc()
    in_maps = make_in_maps(x, norm_w, norm_b, qkv_w, qkv_b, proj_w, proj_b)
    res = run_bass_kernel_spmd(nc, in_maps, core_ids=list(range(b_sz)))
    out = np.stack([r["y"] for r in res.results], axis=0)
    return out.reshape(b_sz, C, h, w)
